# revision 20
# baseline (speedup 1.0000x reference)
"""BertBidaf attention-flow kernel for 8 TRN2 NeuronCores — v3.

Sharding: data-parallel over batch (B=16 -> 2 batches per core); weights
replicated.

The device computes the attention-heavy ~98% of FLOPs: the trilinear
similarity matmul (with the c2q / c*c2q contraction terms riding as 128
extra rhs columns P0/P1), the row softmax, the fused attention
reductions for terms 2+3, and the row-max statistics (nrm, cwc) that
define the q2c attention weights. The remaining rank-1 projections
(q2c = b_att @ c, c @ (W1 + W4*q2c)) are tiny (~2% of FLOPs) and are
folded into the host post-processing together with the final row
masking — this removes the second (row-major) copy of `c` and the whole
q2c/term1 device tail: per-core DMA drops 8.2MB -> 4.7MB and the device
graph collapses to mm1 + 3 short per-tile epilogues per batch.

Per-batch device graph:
  mm1:  ps[t] [128, 193] (t = 3 c-row tiles) = rank-3 bias matmul
        (q-side biases + both sequence masks + q@W2+b_out on the P
        columns, as 3 host-built contraction rows) + 16 accumulating
        chunk matmuls (stationary = cT chunks following the DMA
        wavefront, moving = host-packed qwx [q*w_cq | w_c | q*W3]).
  per tile: nrm = -rowmax(s) (DVE); e = exp(s+nrm) with denominator
        accumulator (Scalar); t23 = (e . P) row-reduce (DVE) * 1/den;
        [t23 | nrm | cwc] written straight into the out tile; one 2KB
        DMA per tile so the tail only waits on the last tile.
Host post: m = cwc - nrm; b_att = softmax(m); q2c = b_att @ c;
        out = c @ (W1 + W4*q2c) + t23 ; masked rows -> -1e12.
"""

import numpy as np
import ml_dtypes

B, C, Q, D = 16, 384, 64, 2048
NCORES = 8
BPC = B // NCORES  # batches per core
NCH = D // 128     # 16 d-chunks
NW = 193           # mm1 rhs width: 64 s-cols + 1 w_c col + 2x64 P-cols
NEG = np.float32(-1e12)
BF16 = ml_dtypes.bfloat16

_cache = {}


def _build_nc():
    import concourse.bass as bass
    import concourse.bacc as bacc
    import concourse.tile as tile
    from concourse import mybir

    f32 = mybir.dt.float32
    bf16 = mybir.dt.bfloat16
    Ax = mybir.AxisListType.X
    Exp = mybir.ActivationFunctionType.Exp
    mul_op = mybir.AluOpType.mult
    add_op = mybir.AluOpType.add
    max_op = mybir.AluOpType.max

    nc = bacc.Bacc("TRN2", target_bir_lowering=False, debug=False)

    cT = nc.declare_dram_parameter("cT", [BPC, 128, NCH, C], bf16,
                                   isOutput=False)
    qwx = nc.declare_dram_parameter("qwx", [BPC, 128, NCH, NW], bf16,
                                    isOutput=False)
    # bias2[:, b, 0:193] = contraction rows (qs+QW2b / low-mask / hi-low)
    # bias2[:, b, 193:577] = stationary cols (ones / ones / rowind)
    bias2 = nc.declare_dram_parameter("bias2", [3, BPC, NW + C], bf16,
                                      isOutput=False)
    # outv[b] = [128, 3, 4] f32: [t23_0 t23_1 nrm cwc] per c-row tile
    outv = nc.declare_dram_parameter("outv", [BPC, 128, 12], f32,
                                     isOutput=True)

    with tile.TileContext(nc) as tc:
        with tc.tile_pool(name="io", bufs=1) as iop, \
             tc.tile_pool(name="wk", bufs=1) as wp, \
             tc.tile_pool(name="ps", bufs=1, space="PSUM") as psp:

            # ---- input loads; the two HWDGE rings are balanced and
            # ordered so arrival order matches mm1 consumption order:
            #   sync ring:   bias2, qwx(b0)lo, cT(b0)p0-p1, cT(b1)p0-p1
            #                (+ out tiles)
            #   scalar ring: qwx(b0)hi, cT(b0)p2-p3, qwx(b1), cT(b1)p2-p3
            b2_sb = iop.tile([3, BPC, NW + C], bf16, tag="bias2")
            nc.scalar.dma_start(out=b2_sb, in_=bias2[:, :, :])
            qwx_sb = []
            cT_sb = [[], []]
            for b in range(BPC):
                tq = iop.tile([128, NCH, NW], bf16, tag=f"qwx{b}")
                qwx_sb.append(tq)
                cT_sb[b] = [iop.tile([128, 4, C], bf16, tag=f"cT{b}p{h}",
                                     name=f"cT{b}p{h}")
                            for h in range(4)]
            nc.scalar.dma_start(out=qwx_sb[0][:, 0:8, :], in_=qwx[0, :, 0:8, :])
            nc.scalar.dma_start(out=qwx_sb[0][:, 8:16, :],
                                in_=qwx[0, :, 8:16, :])
            for h in range(4):
                eng = nc.scalar if h == 3 else nc.sync
                eng.dma_start(out=cT_sb[0][h], in_=cT[0, :, 4 * h:4 * h + 4, :])
            nc.scalar.dma_start(out=qwx_sb[1][:, :, :], in_=qwx[1, :, :, :])
            for h in range(4):
                eng = nc.scalar if h == 3 else nc.sync
                eng.dma_start(out=cT_sb[1][h], in_=cT[1, :, 4 * h:4 * h + 4, :])

            # ---- PE warmup: repeat the (tiny-input) bias matmul into a
            # scratch PSUM bank so the tensor engine's p-state is fully
            # ramped before the real chunk matmuls begin ----
            warm = psp.tile([128, NW], f32, tag="warm")
            for _ in range(34):
                nc.tensor.matmul(warm, b2_sb[:, 0, NW:NW + 128],
                                 b2_sb[:, 0, 0:NW], start=True, stop=True)

            # ---- mm1 for both batches (PE streams behind the DMA
            # wavefront; bias matmuls first so PE starts early) ----
            ps = [[], []]
            for b in range(BPC):
                for t in range(3):
                    p = psp.tile([128, NW], f32, tag=f"ps{b}{t}")
                    nc.tensor.matmul(
                        p, b2_sb[:, b, NW + 128 * t:NW + 128 * (t + 1)],
                        b2_sb[:, b, 0:NW], start=True, stop=False)
                    ps[b].append(p)
                for ch in range(NCH):
                    for t in range(3):
                        nc.tensor.matmul(
                            ps[b][t],
                            cT_sb[b][ch // 4][:, ch % 4,
                                              128 * t:128 * (t + 1)],
                            qwx_sb[b][:, ch, :],
                            start=False, stop=(ch == NCH - 1))

            # ---- per-tile epilogue: softmax stats + fused t23 ----
            for b in range(BPC):
                ov = wp.tile([128, 3, 4], f32, tag="ov", bufs=2)
                for t in range(3):
                    nc.vector.tensor_reduce(
                        out=ov[:, t, 2:3], in_=ps[b][t][:, 0:64], axis=Ax,
                        op=max_op, negate=True)
                    e = wp.tile([128, 64], f32, tag="e", bufs=2)
                    aux = wp.tile([128, 4], f32, tag="aux", bufs=2)
                    nc.scalar.activation(e, ps[b][t][:, 0:64], Exp,
                                         bias=ov[:, t, 2:3], scale=1.0,
                                         accum_out=aux[:, 0:1])
                    nc.scalar.copy(ov[:, t, 3:4], ps[b][t][:, 64:65])
                    scr = wp.tile([128, 2, 64], f32, tag="scr", bufs=2)
                    e_dup = bass.AP(tensor=e.tensor, offset=e.offset,
                                    ap=[e.ap[0], [0, 2], e.ap[1]])
                    nc.vector.tensor_tensor(
                        out=scr,
                        in0=ps[b][t][:, 65:193].rearrange(
                            "p (j i) -> p j i", j=2),
                        in1=e_dup, op=mul_op)
                    nc.vector.tensor_reduce(out=aux[:, 2:4], in_=scr,
                                            axis=Ax, op=add_op)
                    nc.vector.reciprocal(aux[:, 1:2], aux[:, 0:1])
                    nc.vector.tensor_scalar_mul(ov[:, t, 0:2], aux[:, 2:4],
                                                aux[:, 1:2])
                nc.sync.dma_start(out=outv[b, :, :],
                                  in_=ov.rearrange("p a b -> p (a b)"))

    nc.finalize()
    return nc


def _get_nc():
    if "nc" not in _cache:
        _cache["nc"] = _build_nc()
    return _cache["nc"]


def _prep_host(c, q, c_len, q_len, w_c, b_c, w_q, b_q, w_cq, b_cq, W_out,
               b_out):
    """Build per-core device input maps (host-side layout/masking prep)."""
    c = np.asarray(c, np.float32)
    q = np.asarray(q, np.float32)
    c_len = np.asarray(c_len).astype(np.int64)
    q_len = np.asarray(q_len).astype(np.int64)
    w_c = np.asarray(w_c, np.float32)
    w_q = np.asarray(w_q, np.float32)
    w_cq = np.asarray(w_cq, np.float32)
    W_out = np.asarray(W_out, np.float32)
    b_out = np.asarray(b_out, np.float32)
    b_sum = float(np.asarray(b_c, np.float32) + np.asarray(b_q, np.float32)
                  + np.asarray(b_cq, np.float32))

    Mv = np.float32(BF16(-1e12))
    iq = np.arange(Q)
    W2 = W_out[D:2 * D]       # [D, 2] (x = [c, c2q, c*c2q, c*q2c])
    W3 = W_out[2 * D:3 * D]

    in_maps = []
    for core in range(NCORES):
        bs = [BPC * core + i for i in range(BPC)]
        cTm = np.empty((BPC, 128, NCH, C), BF16)
        qwxm = np.empty((BPC, 128, NCH, NW), BF16)
        b2 = np.zeros((3, BPC, NW + C), BF16)
        for i, bidx in enumerate(bs):
            cTm[i] = c[bidx].T.reshape(NCH, 128, C).transpose(1, 0, 2) \
                .astype(BF16)
            qb = q[bidx]
            qT = qb.T                             # [D, Q]
            blk = np.empty((D, NW), np.float32)
            blk[:, 0:64] = qT * w_cq[:, None]
            blk[:, 64] = w_c
            blk[:, 65:129] = qT * W3[:, 0:1]
            blk[:, 129:193] = qT * W3[:, 1:2]
            qwxm[i] = blk.reshape(NCH, 128, NW).transpose(1, 0, 2) \
                .astype(BF16)
            qs = qb @ w_q + b_sum
            low = np.where(iq >= q_len[bidx], Mv, np.float32(0))
            hi = np.where((iq < Q - 1) | (iq >= q_len[bidx]), Mv,
                          np.float32(0))
            QW2b = qb @ W2 + b_out[None, :]
            b2[0, i, 0:64] = qs.astype(BF16)
            b2[0, i, 65:129] = QW2b[:, 0].astype(BF16)
            b2[0, i, 129:193] = QW2b[:, 1].astype(BF16)
            b2[1, i, 0:64] = low.astype(BF16)
            b2[2, i, 0:64] = (hi - low).astype(BF16)
            b2[0, i, NW:NW + C] = BF16(1)
            b2[1, i, NW:NW + C] = BF16(1)
            b2[2, i, NW:NW + C] = (np.arange(C) >= c_len[bidx]) \
                .astype(np.float32).astype(BF16)
        in_maps.append(dict(cT=cTm, qwx=qwxm, bias2=b2))
    return in_maps, (c, c_len, W_out)


def kernel(**inputs):
    from concourse.bass_utils import run_bass_kernel_spmd

    nc = _get_nc()
    in_maps, (c, c_len, W_out) = _prep_host(**inputs)
    res = run_bass_kernel_spmd(nc, in_maps, core_ids=list(range(NCORES)))
    _cache["last_results"] = res

    W1 = W_out[0:D]          # [D, 2]
    W4 = W_out[3 * D:4 * D]

    out = np.empty((B, C, 2), np.float32)
    for core in range(NCORES):
        o = res.results[core]["outv"].reshape(BPC, 128, 3, 4)
        for i in range(BPC):
            bidx = BPC * core + i
            t23 = o[i, :, :, 0:2].transpose(1, 0, 2).reshape(C, 2)
            nrm = o[i, :, :, 2].T.reshape(C)
            cwc = o[i, :, :, 3].T.reshape(C)
            m = cwc - nrm
            eb = np.exp(m - m.max())
            b_att = (eb / eb.sum()).astype(np.float32)
            q2c = b_att @ c[bidx]                       # [D]
            w14 = W1 + W4 * q2c[:, None]                # [D, 2]
            out[bidx] = c[bidx] @ w14 + t23

    rows = np.arange(C)[None, :]
    row_mask = (rows >= c_len[:, None]) & (rows < C - 1)
    out0 = np.where(row_mask, NEG, out[..., 0])
    out1 = np.where(row_mask, NEG, out[..., 1])
    return out0, out1


# revision 22
# speedup vs baseline: 1.1931x; 1.1931x over previous
"""BertBidaf attention-flow kernel for 8 TRN2 NeuronCores — v3.

Sharding: data-parallel over batch (B=16 -> 2 batches per core); weights
replicated.

The device computes the attention-heavy ~98% of FLOPs: the trilinear
similarity matmul (with the c2q / c*c2q contraction terms riding as 128
extra rhs columns P0/P1), the row softmax, the fused attention
reductions for terms 2+3, and the row-max statistics (nrm, cwc) that
define the q2c attention weights. The remaining rank-1 projections
(q2c = b_att @ c, c @ (W1 + W4*q2c)) are tiny (~2% of FLOPs) and are
folded into the host post-processing together with the final row
masking — this removes the second (row-major) copy of `c` and the whole
q2c/term1 device tail: per-core DMA drops 8.2MB -> 4.7MB and the device
graph collapses to mm1 + 3 short per-tile epilogues per batch.

Per-batch device graph:
  mm1:  ps[t] [128, 193] (t = 3 c-row tiles) = rank-3 bias matmul
        (q-side biases + both sequence masks + q@W2+b_out on the P
        columns, as 3 host-built contraction rows) + 16 accumulating
        chunk matmuls (stationary = cT chunks following the DMA
        wavefront, moving = host-packed qwx [q*w_cq | w_c | q*W3]).
  per tile: nrm = -rowmax(s) (DVE); e = exp(s+nrm) with denominator
        accumulator (Scalar); t23 = (e . P) row-reduce (DVE) * 1/den;
        [t23 | nrm | cwc] written straight into the out tile; one 2KB
        DMA per tile so the tail only waits on the last tile.
Host post: m = cwc - nrm; b_att = softmax(m); q2c = b_att @ c;
        out = c @ (W1 + W4*q2c) + t23 ; masked rows -> -1e12.
"""

import numpy as np
import ml_dtypes

B, C, Q, D = 16, 384, 64, 2048
NCORES = 8
BPC = B // NCORES  # batches per core
NCH = D // 128     # 16 d-chunks
NW = 193           # mm1 rhs width: 64 s-cols + 1 w_c col + 2x64 P-cols
NEG = np.float32(-1e12)
BF16 = ml_dtypes.bfloat16

_cache = {}


def _build_nc():
    import concourse.bass as bass
    import concourse.bacc as bacc
    import concourse.tile as tile
    from concourse import mybir

    f32 = mybir.dt.float32
    bf16 = mybir.dt.bfloat16
    Ax = mybir.AxisListType.X
    Exp = mybir.ActivationFunctionType.Exp
    mul_op = mybir.AluOpType.mult
    add_op = mybir.AluOpType.add
    max_op = mybir.AluOpType.max

    nc = bacc.Bacc("TRN2", target_bir_lowering=False, debug=False)

    cT = nc.declare_dram_parameter("cT", [BPC, 128, NCH, C], bf16,
                                   isOutput=False)
    qwx = nc.declare_dram_parameter("qwx", [BPC, 128, NCH, NW], bf16,
                                    isOutput=False)
    # bias2[:, b, 0:193] = contraction rows (qs+QW2b / low-mask / hi-low)
    # bias2[:, b, 193:577] = stationary cols (ones / ones / rowind)
    bias2 = nc.declare_dram_parameter("bias2", [3, BPC, NW + C], bf16,
                                      isOutput=False)
    # outv[b] = [128, 3, 4] f32: [t23_0 t23_1 nrm cwc] per c-row tile
    outv = nc.declare_dram_parameter("outv", [BPC, 128, 12], f32,
                                     isOutput=True)

    with tile.TileContext(nc) as tc:
        with tc.tile_pool(name="io", bufs=1) as iop, \
             tc.tile_pool(name="wk", bufs=1) as wp, \
             tc.tile_pool(name="ps", bufs=1, space="PSUM") as psp:

            # ---- input loads; the two HWDGE rings are balanced and
            # ordered so arrival order matches mm1 consumption order:
            #   sync ring:   bias2, qwx(b0)lo, cT(b0)p0-p1, cT(b1)p0-p1
            #                (+ out tiles)
            #   scalar ring: qwx(b0)hi, cT(b0)p2-p3, qwx(b1), cT(b1)p2-p3
            b2_sb = iop.tile([3, BPC, NW + C], bf16, tag="bias2")
            nc.scalar.dma_start(out=b2_sb, in_=bias2[:, :, :])
            qwx_sb = []
            cT_sb = [None, None]
            for b in range(BPC):
                tq = iop.tile([128, NCH, NW], bf16, tag=f"qwx{b}")
                qwx_sb.append(tq)
                tc_ = iop.tile([128, NCH, C], bf16, tag=f"cT{b}",
                               name=f"cTs{b}")
                cT_sb[b] = tc_
            # scalar ring: bias2, qwx(b0) quarters, qwx(b1), cT(b1) ch8-15
            # sync ring:   cT(b0) ch0-1 / 2-7 / 8-15, cT(b1) ch0-7 (+outs)
            for g in range(4):
                nc.scalar.dma_start(out=qwx_sb[0][:, 4 * g:4 * g + 4, :],
                                    in_=qwx[0, :, 4 * g:4 * g + 4, :])
            nc.sync.dma_start(out=cT_sb[0][:, 0:2, :], in_=cT[0, :, 0:2, :])
            nc.sync.dma_start(out=cT_sb[0][:, 2:8, :], in_=cT[0, :, 2:8, :])
            nc.sync.dma_start(out=cT_sb[0][:, 8:16, :], in_=cT[0, :, 8:16, :])
            nc.scalar.dma_start(out=qwx_sb[1][:, :, :], in_=qwx[1, :, :, :])
            nc.sync.dma_start(out=cT_sb[1][:, 0:8, :], in_=cT[1, :, 0:8, :])
            nc.scalar.dma_start(out=cT_sb[1][:, 8:16, :],
                                in_=cT[1, :, 8:16, :])

            # ---- mm1 for both batches (PE streams behind the DMA
            # wavefront; bias matmuls first so PE starts early) ----
            ps = [[], []]
            for b in range(BPC):
                for t in range(3):
                    p = psp.tile([128, NW], f32, tag=f"ps{b}{t}")
                    nc.tensor.matmul(
                        p, b2_sb[:, b, NW + 128 * t:NW + 128 * (t + 1)],
                        b2_sb[:, b, 0:NW], start=True, stop=False)
                    ps[b].append(p)
                for ch in range(NCH):
                    for t in range(3):
                        nc.tensor.matmul(
                            ps[b][t],
                            cT_sb[b][:, ch, 128 * t:128 * (t + 1)],
                            qwx_sb[b][:, ch, :],
                            start=False, stop=(ch == NCH - 1))

            # ---- per-tile epilogue: softmax stats + fused t23 ----
            for b in range(BPC):
                ov = wp.tile([128, 3, 4], f32, tag="ov", bufs=2)
                for t in range(3):
                    nc.vector.tensor_reduce(
                        out=ov[:, t, 2:3], in_=ps[b][t][:, 0:64], axis=Ax,
                        op=max_op, negate=True)
                    e = wp.tile([128, 64], f32, tag="e", bufs=2)
                    aux = wp.tile([128, 4], f32, tag="aux", bufs=2)
                    nc.scalar.activation(e, ps[b][t][:, 0:64], Exp,
                                         bias=ov[:, t, 2:3], scale=1.0,
                                         accum_out=aux[:, 0:1])
                    nc.scalar.copy(ov[:, t, 3:4], ps[b][t][:, 64:65])
                    scr = wp.tile([128, 2, 64], f32, tag="scr", bufs=2)
                    e_dup = bass.AP(tensor=e.tensor, offset=e.offset,
                                    ap=[e.ap[0], [0, 2], e.ap[1]])
                    nc.vector.tensor_tensor(
                        out=scr,
                        in0=ps[b][t][:, 65:193].rearrange(
                            "p (j i) -> p j i", j=2),
                        in1=e_dup, op=mul_op)
                    nc.vector.tensor_reduce(out=aux[:, 2:4], in_=scr,
                                            axis=Ax, op=add_op)
                    nc.vector.reciprocal(aux[:, 1:2], aux[:, 0:1])
                    nc.vector.tensor_scalar_mul(ov[:, t, 0:2], aux[:, 2:4],
                                                aux[:, 1:2])
                nc.sync.dma_start(out=outv[b, :, :],
                                  in_=ov.rearrange("p a b -> p (a b)"))

    nc.finalize()
    return nc


def _get_nc():
    if "nc" not in _cache:
        _cache["nc"] = _build_nc()
    return _cache["nc"]


def _prep_host(c, q, c_len, q_len, w_c, b_c, w_q, b_q, w_cq, b_cq, W_out,
               b_out):
    """Build per-core device input maps (host-side layout/masking prep)."""
    c = np.asarray(c, np.float32)
    q = np.asarray(q, np.float32)
    c_len = np.asarray(c_len).astype(np.int64)
    q_len = np.asarray(q_len).astype(np.int64)
    w_c = np.asarray(w_c, np.float32)
    w_q = np.asarray(w_q, np.float32)
    w_cq = np.asarray(w_cq, np.float32)
    W_out = np.asarray(W_out, np.float32)
    b_out = np.asarray(b_out, np.float32)
    b_sum = float(np.asarray(b_c, np.float32) + np.asarray(b_q, np.float32)
                  + np.asarray(b_cq, np.float32))

    Mv = np.float32(BF16(-1e12))
    iq = np.arange(Q)
    W2 = W_out[D:2 * D]       # [D, 2] (x = [c, c2q, c*c2q, c*q2c])
    W3 = W_out[2 * D:3 * D]

    in_maps = []
    for core in range(NCORES):
        bs = [BPC * core + i for i in range(BPC)]
        cTm = np.empty((BPC, 128, NCH, C), BF16)
        qwxm = np.empty((BPC, 128, NCH, NW), BF16)
        b2 = np.zeros((3, BPC, NW + C), BF16)
        for i, bidx in enumerate(bs):
            cTm[i] = c[bidx].T.reshape(NCH, 128, C).transpose(1, 0, 2) \
                .astype(BF16)
            qb = q[bidx]
            qT = qb.T                             # [D, Q]
            blk = np.empty((D, NW), np.float32)
            blk[:, 0:64] = qT * w_cq[:, None]
            blk[:, 64] = w_c
            blk[:, 65:129] = qT * W3[:, 0:1]
            blk[:, 129:193] = qT * W3[:, 1:2]
            qwxm[i] = blk.reshape(NCH, 128, NW).transpose(1, 0, 2) \
                .astype(BF16)
            qs = qb @ w_q + b_sum
            low = np.where(iq >= q_len[bidx], Mv, np.float32(0))
            hi = np.where((iq < Q - 1) | (iq >= q_len[bidx]), Mv,
                          np.float32(0))
            QW2b = qb @ W2 + b_out[None, :]
            b2[0, i, 0:64] = qs.astype(BF16)
            b2[0, i, 65:129] = QW2b[:, 0].astype(BF16)
            b2[0, i, 129:193] = QW2b[:, 1].astype(BF16)
            b2[1, i, 0:64] = low.astype(BF16)
            b2[2, i, 0:64] = (hi - low).astype(BF16)
            b2[0, i, NW:NW + C] = BF16(1)
            b2[1, i, NW:NW + C] = BF16(1)
            b2[2, i, NW:NW + C] = (np.arange(C) >= c_len[bidx]) \
                .astype(np.float32).astype(BF16)
        in_maps.append(dict(cT=cTm, qwx=qwxm, bias2=b2))
    return in_maps, (c, c_len, W_out)


def kernel(**inputs):
    from concourse.bass_utils import run_bass_kernel_spmd

    nc = _get_nc()
    in_maps, (c, c_len, W_out) = _prep_host(**inputs)
    res = run_bass_kernel_spmd(nc, in_maps, core_ids=list(range(NCORES)))
    _cache["last_results"] = res

    W1 = W_out[0:D]          # [D, 2]
    W4 = W_out[3 * D:4 * D]

    out = np.empty((B, C, 2), np.float32)
    for core in range(NCORES):
        o = res.results[core]["outv"].reshape(BPC, 128, 3, 4)
        for i in range(BPC):
            bidx = BPC * core + i
            t23 = o[i, :, :, 0:2].transpose(1, 0, 2).reshape(C, 2)
            nrm = o[i, :, :, 2].T.reshape(C)
            cwc = o[i, :, :, 3].T.reshape(C)
            m = cwc - nrm
            eb = np.exp(m - m.max())
            b_att = (eb / eb.sum()).astype(np.float32)
            q2c = b_att @ c[bidx]                       # [D]
            w14 = W1 + W4 * q2c[:, None]                # [D, 2]
            out[bidx] = c[bidx] @ w14 + t23

    rows = np.arange(C)[None, :]
    row_mask = (rows >= c_len[:, None]) & (rows < C - 1)
    out0 = np.where(row_mask, NEG, out[..., 0])
    out1 = np.where(row_mask, NEG, out[..., 1])
    return out0, out1


# revision 23
# speedup vs baseline: 1.2833x; 1.0756x over previous
"""BertBidaf attention-flow kernel for 8 TRN2 NeuronCores — v3.

Sharding: data-parallel over batch (B=16 -> 2 batches per core); weights
replicated.

The device computes the attention-heavy ~98% of FLOPs: the trilinear
similarity matmul (with the c2q / c*c2q contraction terms riding as 128
extra rhs columns P0/P1), the row softmax, the fused attention
reductions for terms 2+3, and the row-max statistics (nrm, cwc) that
define the q2c attention weights. The remaining rank-1 projections
(q2c = b_att @ c, c @ (W1 + W4*q2c)) are tiny (~2% of FLOPs) and are
folded into the host post-processing together with the final row
masking — this removes the second (row-major) copy of `c` and the whole
q2c/term1 device tail: per-core DMA drops 8.2MB -> 4.7MB and the device
graph collapses to mm1 + 3 short per-tile epilogues per batch.

Per-batch device graph:
  mm1:  ps[t] [128, 193] (t = 3 c-row tiles) = rank-3 bias matmul
        (q-side biases + both sequence masks + q@W2+b_out on the P
        columns, as 3 host-built contraction rows) + 16 accumulating
        chunk matmuls (stationary = cT chunks following the DMA
        wavefront, moving = host-packed qwx [q*w_cq | w_c | q*W3]).
  per tile: nrm = -rowmax(s) (DVE); e = exp(s+nrm) with denominator
        accumulator (Scalar); t23 = (e . P) row-reduce (DVE) * 1/den;
        [t23 | nrm | cwc] written straight into the out tile; one 2KB
        DMA per tile so the tail only waits on the last tile.
Host post: m = cwc - nrm; b_att = softmax(m); q2c = b_att @ c;
        out = c @ (W1 + W4*q2c) + t23 ; masked rows -> -1e12.
"""

import numpy as np
import ml_dtypes

B, C, Q, D = 16, 384, 64, 2048
NCORES = 8
BPC = B // NCORES  # batches per core
NCH = D // 128     # 16 d-chunks
NW = 192           # mm1 rhs width: 64 s-cols + 2x64 P-cols (c2q/c*c2q)
NEG = np.float32(-1e12)
BF16 = ml_dtypes.bfloat16

_cache = {}


def _build_nc():
    import concourse.bass as bass
    import concourse.bacc as bacc
    import concourse.tile as tile
    from concourse import mybir

    f32 = mybir.dt.float32
    bf16 = mybir.dt.bfloat16
    Ax = mybir.AxisListType.X
    Exp = mybir.ActivationFunctionType.Exp
    mul_op = mybir.AluOpType.mult
    add_op = mybir.AluOpType.add
    max_op = mybir.AluOpType.max

    nc = bacc.Bacc("TRN2", target_bir_lowering=False, debug=False)

    cT = nc.declare_dram_parameter("cT", [BPC, 128, NCH, C], bf16,
                                   isOutput=False)
    qwx = nc.declare_dram_parameter("qwx", [BPC, 128, NCH, NW], bf16,
                                    isOutput=False)
    # bias2[:, b, 0:193] = contraction rows (qs+QW2b / low-mask / hi-low)
    # bias2[:, b, 193:577] = stationary cols (ones / ones / rowind)
    bias2 = nc.declare_dram_parameter("bias2", [3, BPC, NW + C], bf16,
                                      isOutput=False)
    # outv[b] = [128, 3, 4] f32: [t23_0 t23_1 nrm cwc] per c-row tile
    outv = nc.declare_dram_parameter("outv", [BPC, 128, 12], f32,
                                     isOutput=True)

    with tile.TileContext(nc) as tc:
        with tc.tile_pool(name="io", bufs=1) as iop, \
             tc.tile_pool(name="wk", bufs=1) as wp, \
             tc.tile_pool(name="ps", bufs=1, space="PSUM") as psp:

            # ---- input loads; the two HWDGE rings are balanced and
            # ordered so arrival order matches mm1 consumption order:
            #   sync ring:   bias2, qwx(b0)lo, cT(b0)p0-p1, cT(b1)p0-p1
            #                (+ out tiles)
            #   scalar ring: qwx(b0)hi, cT(b0)p2-p3, qwx(b1), cT(b1)p2-p3
            b2_sb = iop.tile([3, BPC, NW + C], bf16, tag="bias2")
            nc.scalar.dma_start(out=b2_sb, in_=bias2[:, :, :])
            qwx_sb = []
            cT_sb = [None, None]
            for b in range(BPC):
                tq = iop.tile([128, NCH, NW], bf16, tag=f"qwx{b}")
                qwx_sb.append(tq)
                tc_ = iop.tile([128, NCH, C], bf16, tag=f"cT{b}",
                               name=f"cTs{b}")
                cT_sb[b] = tc_
            # scalar ring: bias2, qwx(b0) quarters, qwx(b1), cT(b1) ch8-15
            # sync ring:   cT(b0) ch0-1 / 2-7 / 8-15, cT(b1) ch0-7 (+outs)
            for g in range(4):
                nc.scalar.dma_start(out=qwx_sb[0][:, 4 * g:4 * g + 4, :],
                                    in_=qwx[0, :, 4 * g:4 * g + 4, :])
            nc.sync.dma_start(out=cT_sb[0][:, 0:2, :], in_=cT[0, :, 0:2, :])
            nc.sync.dma_start(out=cT_sb[0][:, 2:8, :], in_=cT[0, :, 2:8, :])
            nc.sync.dma_start(out=cT_sb[0][:, 8:16, :], in_=cT[0, :, 8:16, :])
            nc.scalar.dma_start(out=qwx_sb[1][:, :, :], in_=qwx[1, :, :, :])
            nc.sync.dma_start(out=cT_sb[1][:, 0:8, :], in_=cT[1, :, 0:8, :])
            nc.scalar.dma_start(out=cT_sb[1][:, 8:16, :],
                                in_=cT[1, :, 8:16, :])

            # ---- mm1 for both batches (PE streams behind the DMA
            # wavefront; bias matmuls first so PE starts early) ----
            ps = [[], []]
            for b in range(BPC):
                for t in range(3):
                    p = psp.tile([128, NW], f32, tag=f"ps{b}{t}")
                    nc.tensor.matmul(
                        p, b2_sb[:, b, NW + 128 * t:NW + 128 * (t + 1)],
                        b2_sb[:, b, 0:NW], start=True, stop=False)
                    ps[b].append(p)
                for ch in range(NCH):
                    for t in range(3):
                        nc.tensor.matmul(
                            ps[b][t],
                            cT_sb[b][:, ch, 128 * t:128 * (t + 1)],
                            qwx_sb[b][:, ch, :],
                            start=False, stop=(ch == NCH - 1))

            # ---- per-tile epilogue: softmax stats + fused t23.
            # ov layout [t23raw_0, t23raw_1, nrm, den]; the softmax
            # division and the cwc(=c@w_c) row stats move to the host.
            # All ov writes are DVE so the out DMA waits on one engine.
            for b in range(BPC):
                ov = wp.tile([128, 3, 4], f32, tag="ov", bufs=2)
                for t in range(3):
                    nc.vector.tensor_reduce(
                        out=ov[:, t, 2:3], in_=ps[b][t][:, 0:64], axis=Ax,
                        op=max_op, negate=True)
                    e = wp.tile([128, 64], f32, tag="e", bufs=2)
                    nc.scalar.activation(e, ps[b][t][:, 0:64], Exp,
                                         bias=ov[:, t, 2:3], scale=1.0)
                    nc.vector.tensor_reduce(out=ov[:, t, 3:4], in_=e,
                                            axis=Ax, op=add_op)
                    scr = wp.tile([128, 2, 64], f32, tag="scr", bufs=2)
                    e_dup = bass.AP(tensor=e.tensor, offset=e.offset,
                                    ap=[e.ap[0], [0, 2], e.ap[1]])
                    nc.vector.tensor_tensor(
                        out=scr,
                        in0=ps[b][t][:, 64:192].rearrange(
                            "p (j i) -> p j i", j=2),
                        in1=e_dup, op=mul_op)
                    nc.vector.tensor_reduce(out=ov[:, t, 0:2], in_=scr,
                                            axis=Ax, op=add_op)
                nc.sync.dma_start(out=outv[b, :, :],
                                  in_=ov.rearrange("p a b -> p (a b)"))

    nc.finalize()
    return nc


def _get_nc():
    if "nc" not in _cache:
        _cache["nc"] = _build_nc()
    return _cache["nc"]


def _prep_host(c, q, c_len, q_len, w_c, b_c, w_q, b_q, w_cq, b_cq, W_out,
               b_out):
    """Build per-core device input maps (host-side layout/masking prep)."""
    c = np.asarray(c, np.float32)
    q = np.asarray(q, np.float32)
    c_len = np.asarray(c_len).astype(np.int64)
    q_len = np.asarray(q_len).astype(np.int64)
    w_c = np.asarray(w_c, np.float32)
    w_q = np.asarray(w_q, np.float32)
    w_cq = np.asarray(w_cq, np.float32)
    W_out = np.asarray(W_out, np.float32)
    b_out = np.asarray(b_out, np.float32)
    b_sum = float(np.asarray(b_c, np.float32) + np.asarray(b_q, np.float32)
                  + np.asarray(b_cq, np.float32))

    Mv = np.float32(BF16(-1e12))
    iq = np.arange(Q)
    W2 = W_out[D:2 * D]       # [D, 2] (x = [c, c2q, c*c2q, c*q2c])
    W3 = W_out[2 * D:3 * D]

    in_maps = []
    for core in range(NCORES):
        bs = [BPC * core + i for i in range(BPC)]
        cTm = np.empty((BPC, 128, NCH, C), BF16)
        qwxm = np.empty((BPC, 128, NCH, NW), BF16)
        b2 = np.zeros((3, BPC, NW + C), BF16)
        for i, bidx in enumerate(bs):
            cTm[i] = c[bidx].T.reshape(NCH, 128, C).transpose(1, 0, 2) \
                .astype(BF16)
            qb = q[bidx]
            qT = qb.T                             # [D, Q]
            blk = np.empty((D, NW), np.float32)
            blk[:, 0:64] = qT * w_cq[:, None]
            blk[:, 64:128] = qT * W3[:, 0:1]
            blk[:, 128:192] = qT * W3[:, 1:2]
            qwxm[i] = blk.reshape(NCH, 128, NW).transpose(1, 0, 2) \
                .astype(BF16)
            qs = qb @ w_q + b_sum
            low = np.where(iq >= q_len[bidx], Mv, np.float32(0))
            hi = np.where((iq < Q - 1) | (iq >= q_len[bidx]), Mv,
                          np.float32(0))
            QW2b = qb @ W2 + b_out[None, :]
            b2[0, i, 0:64] = qs.astype(BF16)
            b2[0, i, 64:128] = QW2b[:, 0].astype(BF16)
            b2[0, i, 128:192] = QW2b[:, 1].astype(BF16)
            b2[1, i, 0:64] = low.astype(BF16)
            b2[2, i, 0:64] = (hi - low).astype(BF16)
            b2[0, i, NW:NW + C] = BF16(1)
            b2[1, i, NW:NW + C] = BF16(1)
            b2[2, i, NW:NW + C] = (np.arange(C) >= c_len[bidx]) \
                .astype(np.float32).astype(BF16)
        in_maps.append(dict(cT=cTm, qwx=qwxm, bias2=b2))
    return in_maps, (c, c_len, W_out, w_c)


def kernel(**inputs):
    from concourse.bass_utils import run_bass_kernel_spmd

    nc = _get_nc()
    in_maps, (c, c_len, W_out, w_c) = _prep_host(**inputs)
    res = run_bass_kernel_spmd(nc, in_maps, core_ids=list(range(NCORES)))
    _cache["last_results"] = res

    W1 = W_out[0:D]          # [D, 2]
    W4 = W_out[3 * D:4 * D]

    out = np.empty((B, C, 2), np.float32)
    for core in range(NCORES):
        o = res.results[core]["outv"].reshape(BPC, 128, 3, 4)
        for i in range(BPC):
            bidx = BPC * core + i
            den = o[i, :, :, 3].T.reshape(C)
            t23 = o[i, :, :, 0:2].transpose(1, 0, 2).reshape(C, 2) \
                / den[:, None]
            nrm = o[i, :, :, 2].T.reshape(C)
            m = c[bidx] @ w_c - nrm
            eb = np.exp(m - m.max())
            b_att = (eb / eb.sum()).astype(np.float32)
            q2c = b_att @ c[bidx]                       # [D]
            w14 = W1 + W4 * q2c[:, None]                # [D, 2]
            out[bidx] = c[bidx] @ w14 + t23

    rows = np.arange(C)[None, :]
    row_mask = (rows >= c_len[:, None]) & (rows < C - 1)
    out0 = np.where(row_mask, NEG, out[..., 0])
    out1 = np.where(row_mask, NEG, out[..., 1])
    return out0, out1


# revision 24
# speedup vs baseline: 1.2876x; 1.0033x over previous
"""BertBidaf attention-flow kernel for 8 TRN2 NeuronCores — v3.

Sharding: data-parallel over batch (B=16 -> 2 batches per core); weights
replicated.

The device computes the attention-heavy ~98% of FLOPs: the trilinear
similarity matmul (with the c2q / c*c2q contraction terms riding as 128
extra rhs columns P0/P1), the row softmax, the fused attention
reductions for terms 2+3, and the row-max statistics (nrm, cwc) that
define the q2c attention weights. The remaining rank-1 projections
(q2c = b_att @ c, c @ (W1 + W4*q2c)) are tiny (~2% of FLOPs) and are
folded into the host post-processing together with the final row
masking — this removes the second (row-major) copy of `c` and the whole
q2c/term1 device tail: per-core DMA drops 8.2MB -> 4.7MB and the device
graph collapses to mm1 + 3 short per-tile epilogues per batch.

Per-batch device graph:
  mm1:  ps[t] [128, 193] (t = 3 c-row tiles) = rank-3 bias matmul
        (q-side biases + both sequence masks + q@W2+b_out on the P
        columns, as 3 host-built contraction rows) + 16 accumulating
        chunk matmuls (stationary = cT chunks following the DMA
        wavefront, moving = host-packed qwx [q*w_cq | w_c | q*W3]).
  per tile: nrm = -rowmax(s) (DVE); e = exp(s+nrm) with denominator
        accumulator (Scalar); t23 = (e . P) row-reduce (DVE) * 1/den;
        [t23 | nrm | cwc] written straight into the out tile; one 2KB
        DMA per tile so the tail only waits on the last tile.
Host post: m = cwc - nrm; b_att = softmax(m); q2c = b_att @ c;
        out = c @ (W1 + W4*q2c) + t23 ; masked rows -> -1e12.
"""

import numpy as np
import ml_dtypes

B, C, Q, D = 16, 384, 64, 2048
NCORES = 8
BPC = B // NCORES  # batches per core
NCH = D // 128     # 16 d-chunks
NW = 192           # mm1 rhs width: 64 s-cols + 2x64 P-cols (c2q/c*c2q)
NEG = np.float32(-1e12)
BF16 = ml_dtypes.bfloat16

_cache = {}


def _build_nc():
    import concourse.bass as bass
    import concourse.bacc as bacc
    import concourse.tile as tile
    from concourse import mybir

    f32 = mybir.dt.float32
    bf16 = mybir.dt.bfloat16
    Ax = mybir.AxisListType.X
    Exp = mybir.ActivationFunctionType.Exp
    mul_op = mybir.AluOpType.mult
    add_op = mybir.AluOpType.add
    max_op = mybir.AluOpType.max

    nc = bacc.Bacc("TRN2", target_bir_lowering=False, debug=False)

    cT = nc.declare_dram_parameter("cT", [BPC, 128, NCH, C], bf16,
                                   isOutput=False)
    qwx = nc.declare_dram_parameter("qwx", [BPC, 128, NCH, NW], bf16,
                                    isOutput=False)
    # bias2[:, b, 0:193] = contraction rows (qs+QW2b / low-mask / hi-low)
    # bias2[:, b, 193:577] = stationary cols (ones / ones / rowind)
    bias2 = nc.declare_dram_parameter("bias2", [3, BPC, NW + C], bf16,
                                      isOutput=False)
    # outv[b] = [128, 3, 4] f32: [t23_0 t23_1 nrm cwc] per c-row tile
    outv = nc.declare_dram_parameter("outv", [BPC, 128, 12], f32,
                                     isOutput=True)

    with tile.TileContext(nc) as tc:
        with tc.tile_pool(name="io", bufs=1) as iop, \
             tc.tile_pool(name="wk", bufs=1) as wp, \
             tc.tile_pool(name="ps", bufs=1, space="PSUM") as psp:

            # ---- input loads; the two HWDGE rings are balanced and
            # ordered so arrival order matches mm1 consumption order:
            #   sync ring:   bias2, qwx(b0)lo, cT(b0)p0-p1, cT(b1)p0-p1
            #                (+ out tiles)
            #   scalar ring: qwx(b0)hi, cT(b0)p2-p3, qwx(b1), cT(b1)p2-p3
            b2_sb = iop.tile([3, BPC, NW + C], bf16, tag="bias2")
            nc.scalar.dma_start(out=b2_sb, in_=bias2[:, :, :])
            qwx_sb = []
            cT_sb = [None, None]
            for b in range(BPC):
                tq = iop.tile([128, NCH, NW], bf16, tag=f"qwx{b}")
                qwx_sb.append(tq)
                tc_ = iop.tile([128, NCH, C], bf16, tag=f"cT{b}",
                               name=f"cTs{b}")
                cT_sb[b] = tc_
            # scalar ring: bias2, qwx(b0) quarters, qwx(b1), cT(b1) ch8-15
            # sync ring:   cT(b0) ch0-1 / 2-7 / 8-15, cT(b1) ch0-7 (+outs)
            for g in range(4):
                nc.scalar.dma_start(out=qwx_sb[0][:, 4 * g:4 * g + 4, :],
                                    in_=qwx[0, :, 4 * g:4 * g + 4, :])
            nc.sync.dma_start(out=cT_sb[0][:, 0:2, :], in_=cT[0, :, 0:2, :])
            nc.sync.dma_start(out=cT_sb[0][:, 2:8, :], in_=cT[0, :, 2:8, :])
            nc.sync.dma_start(out=cT_sb[0][:, 8:16, :], in_=cT[0, :, 8:16, :])
            nc.scalar.dma_start(out=qwx_sb[1][:, :, :], in_=qwx[1, :, :, :])
            nc.sync.dma_start(out=cT_sb[1][:, 0:8, :], in_=cT[1, :, 0:8, :])
            nc.scalar.dma_start(out=cT_sb[1][:, 8:16, :],
                                in_=cT[1, :, 8:16, :])

            # ---- mm1 (bias matmuls first so PE starts early; batch 0
            # chunk-major to stream behind the DMA wavefront, batch 1
            # tile-major so its tiles stop early and their epilogues
            # overlap the remaining matmuls) ----
            ps = [[], []]
            for b in range(BPC):
                for t in range(3):
                    p = psp.tile([128, NW], f32, tag=f"ps{b}{t}")
                    nc.tensor.matmul(
                        p, b2_sb[:, b, NW + 128 * t:NW + 128 * (t + 1)],
                        b2_sb[:, b, 0:NW], start=True, stop=False)
                    ps[b].append(p)
            for ch in range(NCH):
                for t in range(3):
                    nc.tensor.matmul(
                        ps[0][t], cT_sb[0][:, ch, 128 * t:128 * (t + 1)],
                        qwx_sb[0][:, ch, :],
                        start=False, stop=(ch == NCH - 1))
            for t in range(3):
                for ch in range(NCH):
                    nc.tensor.matmul(
                        ps[1][t], cT_sb[1][:, ch, 128 * t:128 * (t + 1)],
                        qwx_sb[1][:, ch, :],
                        start=False, stop=(ch == NCH - 1))

            # ---- per-tile epilogue: softmax stats + fused t23.
            # ov layout [t23raw_0, t23raw_1, nrm, den]; the softmax
            # division and the cwc(=c@w_c) row stats move to the host.
            # All ov writes are DVE so the out DMA waits on one engine.
            for b in range(BPC):
                ov = wp.tile([128, 3, 4], f32, tag="ov", bufs=2)
                for t in range(3):
                    nc.vector.tensor_reduce(
                        out=ov[:, t, 2:3], in_=ps[b][t][:, 0:64], axis=Ax,
                        op=max_op, negate=True)
                    e = wp.tile([128, 64], f32, tag="e", bufs=2)
                    nc.scalar.activation(e, ps[b][t][:, 0:64], Exp,
                                         bias=ov[:, t, 2:3], scale=1.0)
                    nc.vector.tensor_reduce(out=ov[:, t, 3:4], in_=e,
                                            axis=Ax, op=add_op)
                    scr = wp.tile([128, 2, 64], f32, tag="scr", bufs=2)
                    e_dup = bass.AP(tensor=e.tensor, offset=e.offset,
                                    ap=[e.ap[0], [0, 2], e.ap[1]])
                    nc.vector.tensor_tensor(
                        out=scr,
                        in0=ps[b][t][:, 64:192].rearrange(
                            "p (j i) -> p j i", j=2),
                        in1=e_dup, op=mul_op)
                    nc.vector.tensor_reduce(out=ov[:, t, 0:2], in_=scr,
                                            axis=Ax, op=add_op)
                nc.sync.dma_start(out=outv[b, :, :],
                                  in_=ov.rearrange("p a b -> p (a b)"))

    nc.finalize()
    return nc


def _get_nc():
    if "nc" not in _cache:
        _cache["nc"] = _build_nc()
    return _cache["nc"]


def _prep_host(c, q, c_len, q_len, w_c, b_c, w_q, b_q, w_cq, b_cq, W_out,
               b_out):
    """Build per-core device input maps (host-side layout/masking prep)."""
    c = np.asarray(c, np.float32)
    q = np.asarray(q, np.float32)
    c_len = np.asarray(c_len).astype(np.int64)
    q_len = np.asarray(q_len).astype(np.int64)
    w_c = np.asarray(w_c, np.float32)
    w_q = np.asarray(w_q, np.float32)
    w_cq = np.asarray(w_cq, np.float32)
    W_out = np.asarray(W_out, np.float32)
    b_out = np.asarray(b_out, np.float32)
    b_sum = float(np.asarray(b_c, np.float32) + np.asarray(b_q, np.float32)
                  + np.asarray(b_cq, np.float32))

    Mv = np.float32(BF16(-1e12))
    iq = np.arange(Q)
    W2 = W_out[D:2 * D]       # [D, 2] (x = [c, c2q, c*c2q, c*q2c])
    W3 = W_out[2 * D:3 * D]

    in_maps = []
    for core in range(NCORES):
        bs = [BPC * core + i for i in range(BPC)]
        cTm = np.empty((BPC, 128, NCH, C), BF16)
        qwxm = np.empty((BPC, 128, NCH, NW), BF16)
        b2 = np.zeros((3, BPC, NW + C), BF16)
        for i, bidx in enumerate(bs):
            cTm[i] = c[bidx].T.reshape(NCH, 128, C).transpose(1, 0, 2) \
                .astype(BF16)
            qb = q[bidx]
            qT = qb.T                             # [D, Q]
            blk = np.empty((D, NW), np.float32)
            blk[:, 0:64] = qT * w_cq[:, None]
            blk[:, 64:128] = qT * W3[:, 0:1]
            blk[:, 128:192] = qT * W3[:, 1:2]
            qwxm[i] = blk.reshape(NCH, 128, NW).transpose(1, 0, 2) \
                .astype(BF16)
            qs = qb @ w_q + b_sum
            low = np.where(iq >= q_len[bidx], Mv, np.float32(0))
            hi = np.where((iq < Q - 1) | (iq >= q_len[bidx]), Mv,
                          np.float32(0))
            QW2b = qb @ W2 + b_out[None, :]
            b2[0, i, 0:64] = qs.astype(BF16)
            b2[0, i, 64:128] = QW2b[:, 0].astype(BF16)
            b2[0, i, 128:192] = QW2b[:, 1].astype(BF16)
            b2[1, i, 0:64] = low.astype(BF16)
            b2[2, i, 0:64] = (hi - low).astype(BF16)
            b2[0, i, NW:NW + C] = BF16(1)
            b2[1, i, NW:NW + C] = BF16(1)
            b2[2, i, NW:NW + C] = (np.arange(C) >= c_len[bidx]) \
                .astype(np.float32).astype(BF16)
        in_maps.append(dict(cT=cTm, qwx=qwxm, bias2=b2))
    return in_maps, (c, c_len, W_out, w_c)


def kernel(**inputs):
    from concourse.bass_utils import run_bass_kernel_spmd

    nc = _get_nc()
    in_maps, (c, c_len, W_out, w_c) = _prep_host(**inputs)
    res = run_bass_kernel_spmd(nc, in_maps, core_ids=list(range(NCORES)))
    _cache["last_results"] = res

    W1 = W_out[0:D]          # [D, 2]
    W4 = W_out[3 * D:4 * D]

    out = np.empty((B, C, 2), np.float32)
    for core in range(NCORES):
        o = res.results[core]["outv"].reshape(BPC, 128, 3, 4)
        for i in range(BPC):
            bidx = BPC * core + i
            den = o[i, :, :, 3].T.reshape(C)
            t23 = o[i, :, :, 0:2].transpose(1, 0, 2).reshape(C, 2) \
                / den[:, None]
            nrm = o[i, :, :, 2].T.reshape(C)
            m = c[bidx] @ w_c - nrm
            eb = np.exp(m - m.max())
            b_att = (eb / eb.sum()).astype(np.float32)
            q2c = b_att @ c[bidx]                       # [D]
            w14 = W1 + W4 * q2c[:, None]                # [D, 2]
            out[bidx] = c[bidx] @ w14 + t23

    rows = np.arange(C)[None, :]
    row_mask = (rows >= c_len[:, None]) & (rows < C - 1)
    out0 = np.where(row_mask, NEG, out[..., 0])
    out1 = np.where(row_mask, NEG, out[..., 1])
    return out0, out1
